# revision 15
# baseline (speedup 1.0000x reference)
"""Multi-head attention (B=2, N=2048, d_model=1024, 16 heads x 64) on 8
Trainium2 NeuronCores.

Sharding: batch x head-group. Core c handles batch b = c//4 and heads
4*(c%4) .. 4*(c%4)+3. Projection weights are column-sliced (rows for Wo) so
each core computes q/k/v projections only for its 4 heads, full attention
for those heads, and a partial output projection. The host sums the four
partial outputs per batch (tensor-parallel reduce on to_out) and adds bo.

Device kernel (per core), matmuls in fp32r (rne-11 mantissa):
  qT/kT : projections producing [head-dim, seq] (lhsT = W chunk)
  v     : natural [seq, head-dim] with a ones column folded in (M=65)
  ST    : k^T q per head -> scores^T [keys, queries]; K=64 row-tile PAIRS
          (two heads concurrently on PE tiles T0/T8)
  E     : exp(ST * scale) via ScalarE eviction PSUM->SBUF (the wall:
          ~1 elem/lane/cycle regardless of dtype)
  AV    : [v|ones]^T @ E -> [65, 512]: rows 0-63 = O^T, row 64 = denom
  norm  : one batched reciprocal per query block, denom broadcast via a
          K=4 pattern matmul, DVE multiply into O^T
  out   : O^T-as-lhsT @ Wo slice -> partial [2048, 1024]
"""

import ml_dtypes
import numpy as np

import concourse.mybir as mybir
import concourse.tile as tile
from concourse import bacc
from concourse import bass_utils
from concourse.tile_rust import add_dep_helper

F32 = mybir.dt.float32
BF16 = mybir.dt.bfloat16
F32R = mybir.dt.float32r
E_DT = mybir.dt.bfloat16
EXP = mybir.ActivationFunctionType.Exp

B = 2
N = 2048
D_MODEL = 1024
NHEAD = 16
DIM_HEAD = 64
SCALE = DIM_HEAD ** (-0.5)
N_CORES = 8
HEADS_PER_CORE = 4          # 2 pairs
INNER = HEADS_PER_CORE * DIM_HEAD  # 256

QB = 512                    # query block
N_QB = N // QB              # 4
N_KC = N // 128             # 16 key chunks


def build_nc():
    nc = bacc.Bacc("TRN2", target_bir_lowering=False, debug=False,
                   num_devices=N_CORES)
    xqt = nc.dram_tensor("xqt", [D_MODEL, N], BF16, kind="ExternalInput").ap()
    xkt = nc.dram_tensor("xkt", [D_MODEL, N], BF16, kind="ExternalInput").ap()
    xvt = nc.dram_tensor("xvt", [D_MODEL, N], BF16, kind="ExternalInput").ap()
    wq = nc.dram_tensor("wq", [D_MODEL, INNER], BF16, kind="ExternalInput").ap()
    wk = nc.dram_tensor("wk", [D_MODEL, INNER], BF16, kind="ExternalInput").ap()
    wv = nc.dram_tensor("wv", [D_MODEL, INNER], BF16, kind="ExternalInput").ap()
    wo = nc.dram_tensor("wo", [INNER, D_MODEL], BF16, kind="ExternalInput").ap()
    vones = nc.dram_tensor("vones", [128, N_KC, HEADS_PER_CORE, 1], BF16,
                           kind="ExternalInput").ap()
    # bc pattern: pat4[k, p, m] = 1 where head k owns output rows m in pair p
    pat4 = nc.dram_tensor("pat4", [128, 2, 128], BF16, kind="ExternalInput").ap()
    out = nc.dram_tensor("out", [N, D_MODEL], F32, kind="ExternalOutput").ap()

    with tile.TileContext(nc) as tc:
        with (
            tc.tile_pool(name="wpool", bufs=1) as wpool,
            tc.tile_pool(name="persist", bufs=1) as persist,
            tc.tile_pool(name="xin", bufs=3) as xin,
            tc.tile_pool(name="ering", bufs=9) as ering,
            tc.tile_pool(name="stage", bufs=3) as stage,
            tc.tile_pool(name="ps_st", bufs=2, space="PSUM") as ps_st,
            tc.tile_pool(name="ps_av", bufs=1, space="PSUM") as ps_av,
            tc.tile_pool(name="ps_misc", bufs=2, space="PSUM") as ps_misc,
        ):
            # ---- weights on sync queue, ordered by first use ----
            wk_sb = wpool.tile([128, 8, INNER], BF16)
            wk_r = wk.rearrange("(c p) m -> p c m", p=128)
            nc.sync.dma_start(wk_sb[:, 0:4, :], wk_r[:, 0:4, :])
            nc.gpsimd.dma_start(wk_sb[:, 4:8, :], wk_r[:, 4:8, :])
            wq_sb = wpool.tile([128, 8, INNER], BF16)
            wq_r = wq.rearrange("(c p) m -> p c m", p=128)
            nc.sync.dma_start(wq_sb[:, 0:4, :], wq_r[:, 0:4, :])
            nc.gpsimd.dma_start(wq_sb[:, 4:8, :], wq_r[:, 4:8, :])

            qt_sb = persist.tile([128, 2, N], BF16)
            kt_sb = persist.tile([128, 2, N], BF16)
            v_sb = persist.tile([128, N_KC, HEADS_PER_CORE, DIM_HEAD + 1], BF16)
            ot_sb = persist.tile([128, 2, N], BF16)

            xqt_r = xqt.rearrange("(c p) n -> p c n", p=128)
            xkt_r = xkt.rearrange("(c p) n -> p c n", p=128)
            xvt_r = xvt.rearrange("(c p) n -> p c n", p=128)

            def emit_kt(n):
                ns = slice(n * QB, (n + 1) * QB)
                xk_t = xin.tile([128, 8, QB], BF16, tag="xin", name=f"xk_{n}")
                for cc in range(4):
                    eng = (nc.sync, nc.gpsimd)[cc % 2]
                    eng.dma_start(xk_t[:, 2 * cc:2 * cc + 2, :],
                                  xkt_r[:, 2 * cc:2 * cc + 2, ns])
                for m in range(2):
                    pk = ps_misc.tile([128, QB], F32, tag="mp", name=f"pk{n}{m}")
                    for c in range(8):
                        nc.tensor.matmul(
                            pk[:], wk_sb[:, c, m * 128:(m + 1) * 128],
                            xk_t[:, c, :], start=(c == 0), stop=(c == 7))
                    nc.vector.tensor_copy(kt_sb[:, m, ns], pk[:])

            def emit_qt(n):
                ns = slice(n * QB, (n + 1) * QB)
                xq_t = xin.tile([128, 8, QB], BF16, tag="xin", name=f"xq_{n}")
                for cc in range(4):
                    eng = (nc.sync, nc.gpsimd)[cc % 2]
                    eng.dma_start(xq_t[:, 2 * cc:2 * cc + 2, :],
                                  xqt_r[:, 2 * cc:2 * cc + 2, ns])
                for m in range(2):
                    pq = ps_misc.tile([128, QB], F32, tag="mp", name=f"pq{n}{m}")
                    for c in range(8):
                        nc.tensor.matmul(
                            pq[:], wq_sb[:, c, m * 128:(m + 1) * 128],
                            xq_t[:, c, :], start=(c == 0), stop=(c == 7))
                    nc.vector.tensor_copy(qt_sb[:, m, ns], pq[:])

            def emit_vblock(n):
                ns = slice(n * QB, (n + 1) * QB)
                xv_t = xin.tile([128, 8, QB], BF16, tag="xin", name=f"xv_{n}")
                for cc in range(2):
                    eng = (nc.gpsimd, nc.sync)[cc % 2]
                    eng.dma_start(xv_t[:, 4 * cc:4 * cc + 4, :],
                                  xvt_r[:, 4 * cc:4 * cc + 4, ns])
                for kci in range(4):
                    kc = n * 4 + kci
                    kcs = slice(kci * 128, (kci + 1) * 128)
                    pva = ps_misc.tile([128, INNER], F32, tag="mp",
                                       padded_shape=[128, 512], name=f"pva{kc}")
                    pvb = ps_misc.tile([128, INNER], F32, tag="mp",
                                       padded_shape=[128, 512], name=f"pvb{kc}")
                    for c in range(8):
                        nc.tensor.matmul(
                            pva[:], xv_t[0:64, c, kcs],
                            wv_sb[0:64, c, :], start=(c == 0), stop=(c == 7))
                        nc.tensor.matmul(
                            pvb[:], xv_t[64:128, c, kcs],
                            wv_sb[64:128, c, :], start=(c == 0), stop=(c == 7))
                    va_sb = stage.tile([128, INNER], F32, tag="vasb",
                                       name=f"va{kc}", bufs=2)
                    nc.vector.tensor_copy(va_sb[:], pva[:])
                    nc.vector.scalar_tensor_tensor(
                        v_sb[:, kc, :, 0:DIM_HEAD],
                        pvb[:].rearrange("p (h d) -> p h d", h=HEADS_PER_CORE),
                        1.0,
                        va_sb[:].rearrange("p (h d) -> p h d", h=HEADS_PER_CORE),
                        mybir.AluOpType.mult, mybir.AluOpType.add)

            def emit_outproj_chunk(qb, idx):
                qc = qb * 4 + idx // 2
                dc = idx % 2
                cs = slice(qc * 128, (qc + 1) * 128)
                op = ps_misc.tile([128, 512], F32, tag="mp", name=f"op{qc}{dc}")
                for ic in range(2):
                    nc.tensor.matmul(
                        op[:], ot_sb[:, ic, cs],
                        wo_sb[:, ic, dc * 512:(dc + 1) * 512],
                        start=(ic == 0), stop=(ic == 1))
                o_stage = stage.tile([128, 512], F32, tag="ostage",
                                     name=f"ost{qc}{dc}", bufs=2)
                nc.vector.tensor_copy(o_stage[:], op[:])
                deng = (nc.sync, nc.gpsimd)[dc]
                deng.dma_start(out[cs, dc * 512:(dc + 1) * 512], o_stage[:])

            qb_state = {}

            def emit_st(qb, p, kc):
                qs = slice(qb * QB, (qb + 1) * QB)
                ks = slice(kc * 128, (kc + 1) * 128)
                st = ps_st.tile([128, 1024], F32, tag="st", name=f"st{qb}{p}{kc}")
                mm0 = nc.tensor.matmul(st[:, 0:512], kt_sb[0:64, p, ks],
                                       qt_sb[0:64, p, qs], start=True, stop=True)
                nc.tensor.matmul(st[:, 512:1024], kt_sb[64:128, p, ks],
                                 qt_sb[64:128, p, qs], start=True, stop=True)
                e_t = ering.tile([128, 2, 512], E_DT, tag="e",
                                 name=f"e{qb}{p}{kc}")
                nc.scalar.activation(
                    e_t[:], st[:].rearrange("p (h n) -> p h n", h=2),
                    EXP, scale=float(SCALE))
                return e_t, mm0

            def emit_av(qb, p, kc, avs, e_t):
                mms = []
                for hh in range(2):
                    mms.append(nc.tensor.matmul(
                        avs[hh][0:DIM_HEAD + 1, :],
                        v_sb[:, kc, 2 * p + hh, :], e_t[:, hh, :],
                        start=(kc == 0), stop=(kc == N_KC - 1)))
                return mms

            def evict_pair(qb, p, avs):
                den4 = qb_state[qb]["den4"]
                avsb = []
                for hh in range(2):
                    a_sb = stage.tile([DIM_HEAD + 1, 512], F32, tag="avsb",
                                      name=f"avsb{qb}_{p}_{hh}", bufs=4)
                    nc.vector.tensor_copy(a_sb[:], avs[hh][0:DIM_HEAD + 1, :])
                    k32 = 32 * (2 * p + hh)
                    nc.gpsimd.tensor_copy(den4[k32:k32 + 1, :],
                                          a_sb[DIM_HEAD:DIM_HEAD + 1, :])
                    avsb.append(a_sb)
                return avsb

            def finalize_pair(qb, p, avsb, order_after=None):
                den4 = qb_state[qb]["den4"]
                qs = slice(qb * QB, (qb + 1) * QB)
                rec = stage.tile([128, 512], F32, tag="rec",
                                 name=f"rec{qb}{p}", bufs=2)
                nc.vector.reciprocal_approx_fast(rec[:], den4[:])
                recr = stage.tile([128, 512], BF16, tag="recr",
                                  name=f"recr{qb}{p}", bufs=2)
                nc.vector.tensor_copy(recr[:], rec[:])
                bc = ps_misc.tile([128, 512], F32, tag="mp", name=f"bc{qb}{p}")
                bcmm = nc.tensor.matmul(bc[:], pat_sb[:, p, :], recr[:],
                                        start=True, stop=True)
                if order_after is not None:
                    add_dep_helper(order_after.ins, bcmm.ins, sync=False,
                                   reason="hold bc behind ST stream")
                for hh in range(2):
                    nc.vector.tensor_mul(
                        ot_sb[hh * 64:(hh + 1) * 64, p, qs],
                        avsb[hh][0:DIM_HEAD, :],
                        bc[hh * 64:(hh + 1) * 64, :])


            def begin_qb(qb):
                den4 = stage.tile([128, 512], F32, tag="den4", name=f"den{qb}",
                                  bufs=1)
                nc.vector.memset(den4[:], 1.0)
                qb_state[qb] = dict(den4=den4)

            def new_avs(qb, p):
                return [ps_av.tile([128, 512], F32, tag=f"av{hh}",
                                   name=f"av{hh}_{qb}_{p}")
                        for hh in range(2)]

            def phase_fillers(qb, p):
                f = []
                if qb == 0 and p == 0:
                    f.append((1, lambda: emit_vblock(0)))
                    for n in range(1, N_QB):
                        f.append((4 * n - 1, lambda n=n: (emit_kt(n),
                                                          emit_vblock(n))))
                elif qb == 0 and p == 1:
                    f.append((5, lambda: emit_qt(1)))
                else:
                    prev = qb - 1
                    if p == 0:
                        for g in range(4):
                            f.append(((4, 7, 10, 13)[g],
                                      lambda g=g: emit_outproj_chunk(prev, g)))
                    else:
                        if qb < N_QB - 1:
                            f.append((4, lambda: emit_qt(qb + 1)))
                        for g in range(4):
                            f.append(((2, 6, 9, 12)[g],
                                      lambda g=g: emit_outproj_chunk(prev, 4 + g)))
                return dict(f)

            emit_kt(0)
            emit_qt(0)

            wv_sb = wpool.tile([128, 8, INNER], BF16)
            nc.sync.dma_start(wv_sb[:], wv.rearrange("(c p) m -> p c m", p=128))
            nc.sync.dma_start(v_sb[:, :, :, DIM_HEAD:DIM_HEAD + 1], vones[:])
            wo_sb = wpool.tile([128, 2, D_MODEL], BF16)
            nc.sync.dma_start(wo_sb[:], wo.rearrange("(c p) d -> p c d", p=128))
            pat_sb = wpool.tile([128, 2, 128], BF16)
            nc.sync.dma_start(pat_sb[:], pat4[:])

            AV_LAG = 4
            phases = [(qb, p) for qb in range(N_QB) for p in range(2)]
            pending = None      # (qb, p, avs, [(kc, e_t)...])
            pending_fin = None  # (qb, p, avsb)

            for qb, p in phases:
                if p == 0:
                    begin_qb(qb)
                avs = new_avs(qb, p)
                fillers = phase_fillers(qb, p)
                eq = []
                for kc in range(N_KC):
                    e_t, stmm = emit_st(qb, p, kc)
                    eq.append((kc, e_t))
                    if kc == 0 and pending is not None:
                        pq, pp, pavs, peq = pending
                        first_av = None
                        for pkc, pe_t in peq:
                            mms = emit_av(pq, pp, pkc, pavs, pe_t)
                            if first_av is None:
                                first_av = mms[0]
                        add_dep_helper(first_av.ins, stmm.ins, sync=False,
                                       reason="drain AVs after first ST")
                        pending_fin = (pq, pp, evict_pair(pq, pp, pavs))
                        pending = None
                    if kc == 2 and pending_fin is not None:
                        fq, fp, favsb = pending_fin
                        finalize_pair(fq, fp, favsb, order_after=stmm)
                        pending_fin = None
                    if kc >= AV_LAG:
                        pkc, pe_t = eq[kc - AV_LAG]
                        emit_av(qb, p, pkc, avs, pe_t)
                    if kc in fillers:
                        fillers[kc]()
                pending = (qb, p, avs, eq[N_KC - AV_LAG:])

            pq, pp, pavs, peq = pending
            for pkc, pe_t in peq:
                emit_av(pq, pp, pkc, pavs, pe_t)
            finalize_pair(pq, pp, evict_pair(pq, pp, pavs))
            for idx in range(8):
                emit_outproj_chunk(N_QB - 1, idx)
    nc.compile()
    return nc


_NC_CACHE = None


def _get_nc():
    global _NC_CACHE
    if _NC_CACHE is None:
        _NC_CACHE = build_nc()
    return _NC_CACHE


def _make_pat4():
    pat = np.zeros((128, 2, 128), np.float32)
    for p in range(2):
        for hh in range(2):
            pat[32 * (2 * p + hh), p, hh * 64:(hh + 1) * 64] = 1.0
    return pat.astype(ml_dtypes.bfloat16)


def _bf16(x):
    return np.ascontiguousarray(np.asarray(x, np.float32)).astype(
        ml_dtypes.bfloat16)


def make_in_maps(query, key, value, Wq, Wk, Wv, Wo):
    query = np.asarray(query, np.float32)
    key = np.asarray(key, np.float32)
    value = np.asarray(value, np.float32)
    vones = np.ones((128, N_KC, HEADS_PER_CORE, 1), np.float32).astype(ml_dtypes.bfloat16)
    pat4 = _make_pat4()
    in_maps = []
    for c in range(N_CORES):
        b = c // 4
        hg = c % 4
        cols = slice(hg * INNER, (hg + 1) * INNER)
        in_maps.append({
            "xqt": _bf16(np.asarray(query[b]).T),
            "xkt": _bf16(np.asarray(key[b]).T),
            "xvt": _bf16(np.asarray(value[b]).T),
            "wq": _bf16(np.asarray(Wq[:, cols])),
            "wk": _bf16(np.asarray(Wk[:, cols])),
            "wv": _bf16(np.asarray(Wv[:, cols])),
            "wo": _bf16(np.asarray(Wo[cols, :])),
            "vones": vones,
            "pat4": pat4,
        })
    return in_maps


def kernel(query, key, value, Wq, Wk, Wv, Wo, bo, _trace=False, _trace_cores=None):
    nc = _get_nc()
    in_maps = make_in_maps(query, key, value, Wq, Wk, Wv, Wo)
    res = bass_utils.run_bass_kernel_spmd(
        nc, in_maps, core_ids=list(range(N_CORES)), trace=_trace,
        trace_cores=_trace_cores)
    out = np.zeros((B, N, D_MODEL), np.float32)
    for c in range(N_CORES):
        out[c // 4] += res.results[c]["out"]
    out += np.asarray(bo, np.float32)[None, None, :]
    if _trace:
        return out, res
    return out



# revision 16
# speedup vs baseline: 1.2693x; 1.2693x over previous
"""Multi-head attention (B=2, N=2048, d_model=1024, 16 heads x 64) on 8
Trainium2 NeuronCores.

Sharding: batch x head-group. Core c handles batch b = c//4 and heads
4*(c%4) .. 4*(c%4)+3. Projection weights are column-sliced (rows for Wo) so
each core computes q/k/v projections only for its 4 heads, full attention
for those heads, and a partial output projection. The host sums the four
partial outputs per batch (tensor-parallel reduce on to_out) and adds bo.

Device kernel (per core), matmuls in fp32r (rne-11 mantissa):
  qT/kT : projections producing [head-dim, seq] (lhsT = W chunk)
  v     : natural [seq, head-dim] with a ones column folded in (M=65)
  ST    : k^T q per head -> scores^T [keys, queries]; K=64 row-tile PAIRS
          (two heads concurrently on PE tiles T0/T8)
  E     : exp(ST * scale) via ScalarE eviction PSUM->SBUF (the wall:
          ~1 elem/lane/cycle regardless of dtype)
  AV    : [v|ones]^T @ E -> [65, 512]: rows 0-63 = O^T, row 64 = denom
  norm  : one batched reciprocal per query block, denom broadcast via a
          K=4 pattern matmul, DVE multiply into O^T
  out   : O^T-as-lhsT @ Wo slice -> partial [2048, 1024]
"""

import ml_dtypes
import numpy as np

import concourse.mybir as mybir
import concourse.tile as tile
from concourse import bacc
from concourse import bass_utils
from concourse.tile_rust import add_dep_helper

F32 = mybir.dt.float32
BF16 = mybir.dt.bfloat16
F32R = mybir.dt.float32r
E_DT = mybir.dt.bfloat16
EXP = mybir.ActivationFunctionType.Exp

B = 2
N = 2048
D_MODEL = 1024
NHEAD = 16
DIM_HEAD = 64
SCALE = DIM_HEAD ** (-0.5)
N_CORES = 8
HEADS_PER_CORE = 4          # 2 pairs
INNER = HEADS_PER_CORE * DIM_HEAD  # 256

QB = 512                    # query block
N_QB = N // QB              # 4
N_KC = N // 128             # 16 key chunks


def build_nc():
    nc = bacc.Bacc("TRN2", target_bir_lowering=False, debug=False,
                   num_devices=N_CORES)
    xqt = nc.dram_tensor("xqt", [D_MODEL, N], BF16, kind="ExternalInput").ap()
    xkt = nc.dram_tensor("xkt", [D_MODEL, N], BF16, kind="ExternalInput").ap()
    xvt = nc.dram_tensor("xvt", [D_MODEL, N], BF16, kind="ExternalInput").ap()
    wq = nc.dram_tensor("wq", [D_MODEL, INNER], BF16, kind="ExternalInput").ap()
    wk = nc.dram_tensor("wk", [D_MODEL, INNER], BF16, kind="ExternalInput").ap()
    wv = nc.dram_tensor("wv", [D_MODEL, INNER], BF16, kind="ExternalInput").ap()
    wo = nc.dram_tensor("wo", [INNER, D_MODEL], BF16, kind="ExternalInput").ap()
    vones = nc.dram_tensor("vones", [128, N_KC, HEADS_PER_CORE, 1], BF16,
                           kind="ExternalInput").ap()
    # bc pattern: pat4[k, p, m] = 1 where head k owns output rows m in pair p
    pat4 = nc.dram_tensor("pat4", [128, 2, 128], BF16, kind="ExternalInput").ap()
    out = nc.dram_tensor("out", [N, D_MODEL], F32, kind="ExternalOutput").ap()

    with tile.TileContext(nc) as tc:
        with (
            tc.tile_pool(name="wpool", bufs=1) as wpool,
            tc.tile_pool(name="persist", bufs=1) as persist,
            tc.tile_pool(name="xin", bufs=3) as xin,
            tc.tile_pool(name="ering", bufs=9) as ering,
            tc.tile_pool(name="stage", bufs=3) as stage,
            tc.tile_pool(name="ps_st", bufs=2, space="PSUM") as ps_st,
            tc.tile_pool(name="ps_av", bufs=1, space="PSUM") as ps_av,
            tc.tile_pool(name="ps_misc", bufs=2, space="PSUM") as ps_misc,
        ):
            # ---- weights on sync queue, ordered by first use ----
            wk_sb = wpool.tile([128, 8, INNER], BF16)
            wk_r = wk.rearrange("(c p) m -> p c m", p=128)
            nc.sync.dma_start(wk_sb[:, 0:4, :], wk_r[:, 0:4, :])
            nc.gpsimd.dma_start(wk_sb[:, 4:8, :], wk_r[:, 4:8, :])
            wq_sb = wpool.tile([128, 8, INNER], BF16)
            wq_r = wq.rearrange("(c p) m -> p c m", p=128)
            nc.sync.dma_start(wq_sb[:, 0:4, :], wq_r[:, 0:4, :])
            nc.gpsimd.dma_start(wq_sb[:, 4:8, :], wq_r[:, 4:8, :])

            qt_sb = persist.tile([128, 2, N], BF16)
            kt_sb = persist.tile([128, 2, N], BF16)
            v_sb = persist.tile([128, N_KC, HEADS_PER_CORE, DIM_HEAD + 1], BF16)
            ot_sb = persist.tile([128, 2, N], BF16)

            xqt_r = xqt.rearrange("(c p) n -> p c n", p=128)
            xkt_r = xkt.rearrange("(c p) n -> p c n", p=128)
            xvt_r = xvt.rearrange("(c p) n -> p c n", p=128)

            def emit_kt(n):
                ns = slice(n * QB, (n + 1) * QB)
                xk_t = xin.tile([128, 8, QB], BF16, tag="xin", name=f"xk_{n}")
                for cc in range(4):
                    eng = (nc.sync, nc.gpsimd)[cc % 2]
                    eng.dma_start(xk_t[:, 2 * cc:2 * cc + 2, :],
                                  xkt_r[:, 2 * cc:2 * cc + 2, ns])
                for m in range(2):
                    pk = ps_misc.tile([128, QB], F32, tag="mp", name=f"pk{n}{m}")
                    for c in range(8):
                        nc.tensor.matmul(
                            pk[:], wk_sb[:, c, m * 128:(m + 1) * 128],
                            xk_t[:, c, :], start=(c == 0), stop=(c == 7))
                    nc.vector.tensor_copy(kt_sb[:, m, ns], pk[:])

            def emit_qt(n):
                ns = slice(n * QB, (n + 1) * QB)
                xq_t = xin.tile([128, 8, QB], BF16, tag="xin", name=f"xq_{n}")
                for cc in range(4):
                    eng = (nc.sync, nc.gpsimd)[cc % 2]
                    eng.dma_start(xq_t[:, 2 * cc:2 * cc + 2, :],
                                  xqt_r[:, 2 * cc:2 * cc + 2, ns])
                for m in range(2):
                    pq = ps_misc.tile([128, QB], F32, tag="mp", name=f"pq{n}{m}")
                    for c in range(8):
                        nc.tensor.matmul(
                            pq[:], wq_sb[:, c, m * 128:(m + 1) * 128],
                            xq_t[:, c, :], start=(c == 0), stop=(c == 7))
                    nc.vector.tensor_copy(qt_sb[:, m, ns], pq[:])

            def emit_vblock(n):
                ns = slice(n * QB, (n + 1) * QB)
                xv_t = xin.tile([128, 8, QB], BF16, tag="xin", name=f"xv_{n}")
                for cc in range(2):
                    eng = (nc.gpsimd, nc.sync)[cc % 2]
                    eng.dma_start(xv_t[:, 4 * cc:4 * cc + 4, :],
                                  xvt_r[:, 4 * cc:4 * cc + 4, ns])
                for kci in range(4):
                    kc = n * 4 + kci
                    kcs = slice(kci * 128, (kci + 1) * 128)
                    pva = ps_misc.tile([128, INNER], F32, tag="mp",
                                       padded_shape=[128, 512], name=f"pva{kc}")
                    pvb = ps_misc.tile([128, INNER], F32, tag="mp",
                                       padded_shape=[128, 512], name=f"pvb{kc}")
                    for c in range(8):
                        nc.tensor.matmul(
                            pva[:], xv_t[0:64, c, kcs],
                            wv_sb[0:64, c, :], start=(c == 0), stop=(c == 7))
                        nc.tensor.matmul(
                            pvb[:], xv_t[64:128, c, kcs],
                            wv_sb[64:128, c, :], start=(c == 0), stop=(c == 7))
                    va_sb = stage.tile([128, INNER], F32, tag="vasb",
                                       name=f"va{kc}", bufs=2)
                    nc.vector.tensor_copy(va_sb[:], pva[:])
                    nc.vector.scalar_tensor_tensor(
                        v_sb[:, kc, :, 0:DIM_HEAD],
                        pvb[:].rearrange("p (h d) -> p h d", h=HEADS_PER_CORE),
                        1.0,
                        va_sb[:].rearrange("p (h d) -> p h d", h=HEADS_PER_CORE),
                        mybir.AluOpType.mult, mybir.AluOpType.add)

            def emit_outproj_chunk(qb, idx):
                qc = qb * 4 + idx // 2
                dc = idx % 2
                cs = slice(qc * 128, (qc + 1) * 128)
                op = ps_misc.tile([128, 512], F32, tag="mp", name=f"op{qc}{dc}")
                for ic in range(2):
                    nc.tensor.matmul(
                        op[:], ot_sb[:, ic, cs],
                        wo_sb[:, ic, dc * 512:(dc + 1) * 512],
                        start=(ic == 0), stop=(ic == 1))
                o_stage = stage.tile([128, 512], F32, tag="ostage",
                                     name=f"ost{qc}{dc}", bufs=2)
                nc.vector.tensor_copy(o_stage[:], op[:])
                deng = (nc.sync, nc.gpsimd)[dc]
                deng.dma_start(out[cs, dc * 512:(dc + 1) * 512], o_stage[:])

            qb_state = {}

            def emit_st(qb, p, kc):
                qs = slice(qb * QB, (qb + 1) * QB)
                ks = slice(kc * 128, (kc + 1) * 128)
                st = ps_st.tile([128, 1024], F32, tag="st", name=f"st{qb}{p}{kc}")
                mm0 = nc.tensor.matmul(st[:, 0:512], kt_sb[0:64, p, ks],
                                       qt_sb[0:64, p, qs], start=True, stop=True)
                nc.tensor.matmul(st[:, 512:1024], kt_sb[64:128, p, ks],
                                 qt_sb[64:128, p, qs], start=True, stop=True)
                e_t = ering.tile([128, 2, 512], E_DT, tag="e",
                                 name=f"e{qb}{p}{kc}")
                nc.scalar.activation(
                    e_t[:], st[:].rearrange("p (h n) -> p h n", h=2),
                    EXP, scale=float(SCALE))
                return e_t, mm0

            def emit_av(qb, p, kc, avs, e_t):
                mms = []
                for hh in range(2):
                    mms.append(nc.tensor.matmul(
                        avs[hh][0:DIM_HEAD + 1, :],
                        v_sb[:, kc, 2 * p + hh, :], e_t[:, hh, :],
                        start=(kc == 0), stop=(kc == N_KC - 1)))
                return mms

            def evict_pair(qb, p, avs):
                den4 = qb_state[qb]["den4"]
                avsb = []
                for hh in range(2):
                    a_sb = stage.tile([DIM_HEAD + 1, 512], F32, tag="avsb",
                                      name=f"avsb{qb}_{p}_{hh}", bufs=4)
                    nc.vector.tensor_copy(a_sb[:], avs[hh][0:DIM_HEAD + 1, :])
                    k32 = 32 * (2 * p + hh)
                    nc.vector.tensor_copy(den4[k32:k32 + 1, :],
                                          a_sb[DIM_HEAD:DIM_HEAD + 1, :])
                    avsb.append(a_sb)
                return avsb

            def finalize_pair(qb, p, avsb, order_after=None):
                den4 = qb_state[qb]["den4"]
                qs = slice(qb * QB, (qb + 1) * QB)
                rec = stage.tile([128, 512], F32, tag="rec",
                                 name=f"rec{qb}{p}", bufs=2)
                nc.vector.reciprocal_approx_fast(rec[:], den4[:])
                recr = stage.tile([128, 512], BF16, tag="recr",
                                  name=f"recr{qb}{p}", bufs=2)
                nc.vector.tensor_copy(recr[:], rec[:])
                bc = ps_misc.tile([128, 512], F32, tag="mp", name=f"bc{qb}{p}")
                bcmm = nc.tensor.matmul(bc[:], pat_sb[:, p, :], recr[:],
                                        start=True, stop=True)
                if order_after is not None:
                    add_dep_helper(order_after.ins, bcmm.ins, sync=False,
                                   reason="hold bc behind ST stream")
                for hh in range(2):
                    nc.vector.tensor_mul(
                        ot_sb[hh * 64:(hh + 1) * 64, p, qs],
                        avsb[hh][0:DIM_HEAD, :],
                        bc[hh * 64:(hh + 1) * 64, :])


            def begin_qb(qb):
                den4 = stage.tile([128, 512], F32, tag="den4", name=f"den{qb}",
                                  bufs=1)
                nc.vector.memset(den4[:], 1.0)
                qb_state[qb] = dict(den4=den4)

            def new_avs(qb, p):
                return [ps_av.tile([128, 512], F32, tag=f"av{hh}",
                                   name=f"av{hh}_{qb}_{p}")
                        for hh in range(2)]

            def phase_fillers(qb, p):
                f = []
                if qb == 0 and p == 0:
                    f.append((1, lambda: emit_vblock(0)))
                    for n in range(1, N_QB):
                        f.append((4 * n - 1, lambda n=n: (emit_kt(n),
                                                          emit_vblock(n))))
                elif qb == 0 and p == 1:
                    f.append((5, lambda: emit_qt(1)))
                else:
                    prev = qb - 1
                    if p == 0:
                        for g in range(4):
                            f.append(((4, 7, 10, 13)[g],
                                      lambda g=g: emit_outproj_chunk(prev, g)))
                    else:
                        if qb < N_QB - 1:
                            f.append((4, lambda: emit_qt(qb + 1)))
                        for g in range(4):
                            f.append(((2, 6, 9, 12)[g],
                                      lambda g=g: emit_outproj_chunk(prev, 4 + g)))
                return dict(f)

            emit_kt(0)
            emit_qt(0)

            wv_sb = wpool.tile([128, 8, INNER], BF16)
            nc.sync.dma_start(wv_sb[:], wv.rearrange("(c p) m -> p c m", p=128))
            nc.sync.dma_start(v_sb[:, :, :, DIM_HEAD:DIM_HEAD + 1], vones[:])
            wo_sb = wpool.tile([128, 2, D_MODEL], BF16)
            nc.sync.dma_start(wo_sb[:], wo.rearrange("(c p) d -> p c d", p=128))
            pat_sb = wpool.tile([128, 2, 128], BF16)
            nc.sync.dma_start(pat_sb[:], pat4[:])

            AV_LAG = 4
            phases = [(qb, p) for qb in range(N_QB) for p in range(2)]
            pending = None      # (qb, p, avs, [(kc, e_t)...])
            pending_fin = None  # (qb, p, avsb)

            for qb, p in phases:
                if p == 0:
                    begin_qb(qb)
                avs = new_avs(qb, p)
                fillers = phase_fillers(qb, p)
                eq = []
                for kc in range(N_KC):
                    e_t, stmm = emit_st(qb, p, kc)
                    eq.append((kc, e_t))
                    if kc == 0 and pending is not None:
                        pq, pp, pavs, peq = pending
                        first_av = None
                        for pkc, pe_t in peq:
                            mms = emit_av(pq, pp, pkc, pavs, pe_t)
                            if first_av is None:
                                first_av = mms[0]
                        add_dep_helper(first_av.ins, stmm.ins, sync=False,
                                       reason="drain AVs after first ST")
                        pending_fin = (pq, pp, evict_pair(pq, pp, pavs))
                        pending = None
                    if kc == 2 and pending_fin is not None:
                        fq, fp, favsb = pending_fin
                        finalize_pair(fq, fp, favsb, order_after=stmm)
                        pending_fin = None
                    if kc >= AV_LAG:
                        pkc, pe_t = eq[kc - AV_LAG]
                        emit_av(qb, p, pkc, avs, pe_t)
                    if kc in fillers:
                        fillers[kc]()
                pending = (qb, p, avs, eq[N_KC - AV_LAG:])

            pq, pp, pavs, peq = pending
            for pkc, pe_t in peq:
                emit_av(pq, pp, pkc, pavs, pe_t)
            finalize_pair(pq, pp, evict_pair(pq, pp, pavs))
            for idx in range(8):
                emit_outproj_chunk(N_QB - 1, idx)
    nc.compile()
    return nc


_NC_CACHE = None


def _get_nc():
    global _NC_CACHE
    if _NC_CACHE is None:
        _NC_CACHE = build_nc()
    return _NC_CACHE


def _make_pat4():
    pat = np.zeros((128, 2, 128), np.float32)
    for p in range(2):
        for hh in range(2):
            pat[32 * (2 * p + hh), p, hh * 64:(hh + 1) * 64] = 1.0
    return pat.astype(ml_dtypes.bfloat16)


def _bf16(x):
    return np.ascontiguousarray(np.asarray(x, np.float32)).astype(
        ml_dtypes.bfloat16)


def make_in_maps(query, key, value, Wq, Wk, Wv, Wo):
    query = np.asarray(query, np.float32)
    key = np.asarray(key, np.float32)
    value = np.asarray(value, np.float32)
    vones = np.ones((128, N_KC, HEADS_PER_CORE, 1), np.float32).astype(ml_dtypes.bfloat16)
    pat4 = _make_pat4()
    in_maps = []
    for c in range(N_CORES):
        b = c // 4
        hg = c % 4
        cols = slice(hg * INNER, (hg + 1) * INNER)
        in_maps.append({
            "xqt": _bf16(np.asarray(query[b]).T),
            "xkt": _bf16(np.asarray(key[b]).T),
            "xvt": _bf16(np.asarray(value[b]).T),
            "wq": _bf16(np.asarray(Wq[:, cols])),
            "wk": _bf16(np.asarray(Wk[:, cols])),
            "wv": _bf16(np.asarray(Wv[:, cols])),
            "wo": _bf16(np.asarray(Wo[cols, :])),
            "vones": vones,
            "pat4": pat4,
        })
    return in_maps


def kernel(query, key, value, Wq, Wk, Wv, Wo, bo, _trace=False, _trace_cores=None):
    nc = _get_nc()
    in_maps = make_in_maps(query, key, value, Wq, Wk, Wv, Wo)
    res = bass_utils.run_bass_kernel_spmd(
        nc, in_maps, core_ids=list(range(N_CORES)), trace=_trace,
        trace_cores=_trace_cores)
    out = np.zeros((B, N, D_MODEL), np.float32)
    for c in range(N_CORES):
        out[c // 4] += res.results[c]["out"]
    out += np.asarray(bo, np.float32)[None, None, :]
    if _trace:
        return out, res
    return out



# revision 21
# speedup vs baseline: 1.4020x; 1.1045x over previous
"""Multi-head attention (B=2, N=2048, d_model=1024, 16 heads x 64) on 8
Trainium2 NeuronCores.

Sharding: batch x head-group. Core c handles batch b = c//4 and heads
4*(c%4) .. 4*(c%4)+3. Projection weights are column-sliced (rows for Wo) so
each core computes q/k/v projections only for its 4 heads, full attention
for those heads, and a partial output projection. The host sums the four
partial outputs per batch (tensor-parallel reduce on to_out) and adds bo.

Device kernel (per core), matmuls in fp32r (rne-11 mantissa):
  qT/kT : projections producing [head-dim, seq] (lhsT = W chunk)
  v     : natural [seq, head-dim] with a ones column folded in (M=65)
  ST    : k^T q per head -> scores^T [keys, queries]; K=64 row-tile PAIRS
          (two heads concurrently on PE tiles T0/T8)
  E     : exp(ST * scale) via ScalarE eviction PSUM->SBUF (the wall:
          ~1 elem/lane/cycle regardless of dtype)
  AV    : [v|ones]^T @ E -> [65, 512]: rows 0-63 = O^T, row 64 = denom
  norm  : one batched reciprocal per query block, denom broadcast via a
          K=4 pattern matmul, DVE multiply into O^T
  out   : O^T-as-lhsT @ Wo slice -> partial [2048, 1024]
"""

import ml_dtypes
import numpy as np

import concourse.mybir as mybir
import concourse.tile as tile
from concourse import bacc
from concourse import bass_utils
from concourse.tile_rust import add_dep_helper

F32 = mybir.dt.float32
BF16 = mybir.dt.bfloat16
F32R = mybir.dt.float32r
E_DT = mybir.dt.bfloat16
EXP = mybir.ActivationFunctionType.Exp

B = 2
N = 2048
D_MODEL = 1024
NHEAD = 16
DIM_HEAD = 64
SCALE = DIM_HEAD ** (-0.5)
N_CORES = 8
HEADS_PER_CORE = 4          # 2 pairs
INNER = HEADS_PER_CORE * DIM_HEAD  # 256

QB = 512                    # query block
N_QB = N // QB              # 4
N_KC = N // 128             # 16 key chunks


def build_nc():
    nc = bacc.Bacc("TRN2", target_bir_lowering=False, debug=False,
                   num_devices=N_CORES)
    xqt = nc.dram_tensor("xqt", [D_MODEL, N], BF16, kind="ExternalInput").ap()
    xkt = nc.dram_tensor("xkt", [D_MODEL, N], BF16, kind="ExternalInput").ap()
    xvt = nc.dram_tensor("xvt", [D_MODEL, N], BF16, kind="ExternalInput").ap()
    wq = nc.dram_tensor("wq", [D_MODEL, INNER], BF16, kind="ExternalInput").ap()
    wk = nc.dram_tensor("wk", [D_MODEL, INNER], BF16, kind="ExternalInput").ap()
    wv = nc.dram_tensor("wv", [D_MODEL, INNER], BF16, kind="ExternalInput").ap()
    wo = nc.dram_tensor("wo", [INNER, D_MODEL], BF16, kind="ExternalInput").ap()
    vones = nc.dram_tensor("vones", [128, N_KC, HEADS_PER_CORE, 1], BF16,
                           kind="ExternalInput").ap()
    # bc pattern: pat4[k, p, m] = 1 where head k owns output rows m in pair p
    pat4 = nc.dram_tensor("pat4", [128, 2, 128], BF16, kind="ExternalInput").ap()
    out = nc.dram_tensor("out", [N, D_MODEL], BF16, kind="ExternalOutput").ap()

    with tile.TileContext(nc) as tc:
        with (
            tc.tile_pool(name="wpool", bufs=1) as wpool,
            tc.tile_pool(name="persist", bufs=1) as persist,
            tc.tile_pool(name="xin", bufs=6) as xin,
            tc.tile_pool(name="ering", bufs=9) as ering,
            tc.tile_pool(name="stage", bufs=3) as stage,
            tc.tile_pool(name="ps_st", bufs=2, space="PSUM") as ps_st,
            tc.tile_pool(name="ps_av", bufs=1, space="PSUM") as ps_av,
            tc.tile_pool(name="ps_misc", bufs=2, space="PSUM") as ps_misc,
        ):
            # ---- PE pre-warm: keep HAM busy so real matmuls start at
            # full clock. Dummy matmuls on a tiny scratch tile. ----
            warm_sb = wpool.tile([128, 64], BF16)
            nc.vector.memset(warm_sb[:], 0.0)
            warm_ps = ps_misc.tile([128, 64], F32, tag="mp", name="warmps")
            for wi in range(64):
                nc.tensor.matmul(warm_ps[0:64, 0:64], warm_sb[:, 0:64],
                                 warm_sb[:, 0:64], start=True, stop=True)

            # ---- weights on sync queue, ordered by first use ----
            wk_sb = wpool.tile([128, 8, INNER], BF16)
            wk_r = wk.rearrange("(c p) m -> p c m", p=128)
            nc.sync.dma_start(wk_sb[:, 0:4, :], wk_r[:, 0:4, :])
            nc.gpsimd.dma_start(wk_sb[:, 4:8, :], wk_r[:, 4:8, :])
            wq_sb = wpool.tile([128, 8, INNER], BF16)
            wq_r = wq.rearrange("(c p) m -> p c m", p=128)
            nc.sync.dma_start(wq_sb[:, 0:4, :], wq_r[:, 0:4, :])
            nc.gpsimd.dma_start(wq_sb[:, 4:8, :], wq_r[:, 4:8, :])

            qt_sb = persist.tile([128, 2, N], BF16)
            kt_sb = persist.tile([128, 2, N], BF16)
            v_sb = persist.tile([128, N_KC, HEADS_PER_CORE, DIM_HEAD + 1], BF16)
            ot_sb = persist.tile([128, 2, N], BF16)

            xqt_r = xqt.rearrange("(c p) n -> p c n", p=128)
            xkt_r = xkt.rearrange("(c p) n -> p c n", p=128)
            xvt_r = xvt.rearrange("(c p) n -> p c n", p=128)

            def emit_kdma(n):
                ns = slice(n * QB, (n + 1) * QB)
                xk_t = xin.tile([128, 8, QB], BF16, tag="xin", name=f"xk_{n}")
                for cc in range(4):
                    eng = (nc.sync, nc.gpsimd)[cc % 2]
                    eng.dma_start(xk_t[:, 2 * cc:2 * cc + 2, :],
                                  xkt_r[:, 2 * cc:2 * cc + 2, ns])
                return xk_t

            def emit_kt(n, xk_t=None):
                ns = slice(n * QB, (n + 1) * QB)
                if xk_t is None:
                    xk_t = emit_kdma(n)
                for m in range(2):
                    pk = ps_misc.tile([128, QB], F32, tag="mp", name=f"pk{n}{m}")
                    for c in range(8):
                        nc.tensor.matmul(
                            pk[:], wk_sb[:, c, m * 128:(m + 1) * 128],
                            xk_t[:, c, :], start=(c == 0), stop=(c == 7))
                    nc.vector.tensor_copy(kt_sb[:, m, ns], pk[:])

            def emit_qdma(n):
                ns = slice(n * QB, (n + 1) * QB)
                xq_t = xin.tile([128, 8, QB], BF16, tag="xin", name=f"xq_{n}")
                for cc in range(4):
                    eng = (nc.sync, nc.gpsimd)[cc % 2]
                    eng.dma_start(xq_t[:, 2 * cc:2 * cc + 2, :],
                                  xqt_r[:, 2 * cc:2 * cc + 2, ns])
                return xq_t

            def emit_qt(n, xq_t=None):
                ns = slice(n * QB, (n + 1) * QB)
                if xq_t is None:
                    xq_t = emit_qdma(n)
                for m in range(2):
                    pq = ps_misc.tile([128, QB], F32, tag="mp", name=f"pq{n}{m}")
                    for c in range(8):
                        nc.tensor.matmul(
                            pq[:], wq_sb[:, c, m * 128:(m + 1) * 128],
                            xq_t[:, c, :], start=(c == 0), stop=(c == 7))
                    nc.vector.tensor_copy(qt_sb[:, m, ns], pq[:])

            def emit_vdma(n):
                ns = slice(n * QB, (n + 1) * QB)
                xv_t = xin.tile([128, 8, QB], BF16, tag="xin", name=f"xv_{n}")
                for cc in range(2):
                    eng = (nc.gpsimd, nc.sync)[cc % 2]
                    eng.dma_start(xv_t[:, 4 * cc:4 * cc + 4, :],
                                  xvt_r[:, 4 * cc:4 * cc + 4, ns])
                return xv_t

            def emit_vblock(n, xv_t=None):
                ns = slice(n * QB, (n + 1) * QB)
                if xv_t is None:
                    xv_t = emit_vdma(n)
                for kci in range(4):
                    kc = n * 4 + kci
                    kcs = slice(kci * 128, (kci + 1) * 128)
                    pva = ps_misc.tile([128, INNER], F32, tag="mp",
                                       padded_shape=[128, 512], name=f"pva{kc}")
                    pvb = ps_misc.tile([128, INNER], F32, tag="mp",
                                       padded_shape=[128, 512], name=f"pvb{kc}")
                    for c in range(8):
                        nc.tensor.matmul(
                            pva[:], xv_t[0:64, c, kcs],
                            wv_sb[0:64, c, :], start=(c == 0), stop=(c == 7))
                        nc.tensor.matmul(
                            pvb[:], xv_t[64:128, c, kcs],
                            wv_sb[64:128, c, :], start=(c == 0), stop=(c == 7))
                    va_sb = stage.tile([128, INNER], F32, tag="vasb",
                                       name=f"va{kc}", bufs=2)
                    nc.vector.tensor_copy(va_sb[:], pva[:])
                    nc.vector.scalar_tensor_tensor(
                        v_sb[:, kc, :, 0:DIM_HEAD],
                        pvb[:].rearrange("p (h d) -> p h d", h=HEADS_PER_CORE),
                        1.0,
                        va_sb[:].rearrange("p (h d) -> p h d", h=HEADS_PER_CORE),
                        mybir.AluOpType.mult, mybir.AluOpType.add)

            def emit_outproj_chunk(qb, idx, tail=False):
                qc = qb * 4 + idx // 2
                dc = idx % 2
                cs = slice(qc * 128, (qc + 1) * 128)
                op = ps_misc.tile([128, 512], F32, tag="mp", name=f"op{qc}{dc}")
                for ic in range(2):
                    nc.tensor.matmul(
                        op[:], ot_sb[:, ic, cs],
                        wo_sb[:, ic, dc * 512:(dc + 1) * 512],
                        start=(ic == 0), stop=(ic == 1))
                o_stage = stage.tile([128, 512], BF16, tag="ostage",
                                     name=f"ost{qc}{dc}", bufs=2)
                if tail and (idx % 2 == 0):
                    nc.scalar.copy(o_stage[:], op[:])
                else:
                    nc.vector.tensor_copy(o_stage[:], op[:])
                deng = (nc.sync, nc.gpsimd)[dc]
                deng.dma_start(out[cs, dc * 512:(dc + 1) * 512], o_stage[:])

            qb_state = {}

            def emit_st(qb, p, kc):
                qs = slice(qb * QB, (qb + 1) * QB)
                ks = slice(kc * 128, (kc + 1) * 128)
                st = ps_st.tile([128, 1024], F32, tag="st", name=f"st{qb}{p}{kc}")
                mm0 = nc.tensor.matmul(st[:, 0:512], kt_sb[0:64, p, ks],
                                       qt_sb[0:64, p, qs], start=True, stop=True)
                mm1 = nc.tensor.matmul(st[:, 512:1024], kt_sb[64:128, p, ks],
                                 qt_sb[64:128, p, qs], start=True, stop=True)
                e_t = ering.tile([128, 2, 512], E_DT, tag="e",
                                 name=f"e{qb}{p}{kc}")
                nc.scalar.activation(
                    e_t[:], st[:].rearrange("p (h n) -> p h n", h=2),
                    EXP, scale=float(SCALE))
                return e_t, mm0, mm1

            def emit_av(qb, p, kc, avs, e_t):
                mms = []
                for hh in range(2):
                    mms.append(nc.tensor.matmul(
                        avs[hh][0:DIM_HEAD + 1, :],
                        v_sb[:, kc, 2 * p + hh, :], e_t[:, hh, :],
                        start=(kc == 0), stop=(kc == N_KC - 1)))
                return mms

            def evict_pair(qb, p, avs):
                den4 = qb_state[qb]["den4"]
                avsb = []
                for hh in range(2):
                    a_sb = stage.tile([DIM_HEAD + 1, 512], F32, tag="avsb",
                                      name=f"avsb{qb}_{p}_{hh}", bufs=4)
                    nc.vector.tensor_copy(a_sb[:], avs[hh][0:DIM_HEAD + 1, :])
                    k32 = 32 * (2 * p + hh)
                    nc.vector.tensor_copy(den4[k32:k32 + 1, :],
                                          a_sb[DIM_HEAD:DIM_HEAD + 1, :])
                    avsb.append(a_sb)
                return avsb

            def finalize_pair(qb, p, avsb, order_after=None):
                den4 = qb_state[qb]["den4"]
                qs = slice(qb * QB, (qb + 1) * QB)
                rec = stage.tile([128, 512], F32, tag="rec",
                                 name=f"rec{qb}{p}", bufs=2)
                nc.vector.reciprocal_approx_fast(rec[:], den4[:])
                recr = stage.tile([128, 512], BF16, tag="recr",
                                  name=f"recr{qb}{p}", bufs=2)
                nc.vector.tensor_copy(recr[:], rec[:])
                bc = ps_misc.tile([128, 512], F32, tag="mp", name=f"bc{qb}{p}")
                bcmm = nc.tensor.matmul(bc[:], pat_sb[:, p, :], recr[:],
                                        start=True, stop=True)
                if order_after is not None:
                    add_dep_helper(order_after.ins, bcmm.ins, sync=False,
                                   reason="hold bc behind ST stream")
                for hh in range(2):
                    nc.vector.tensor_mul(
                        ot_sb[hh * 64:(hh + 1) * 64, p, qs],
                        avsb[hh][0:DIM_HEAD, :],
                        bc[hh * 64:(hh + 1) * 64, :])


            def begin_qb(qb):
                den4 = stage.tile([128, 512], F32, tag="den4", name=f"den{qb}",
                                  bufs=1)
                nc.gpsimd.memset(den4[:], 1.0)
                qb_state[qb] = dict(den4=den4)

            def new_avs(qb, p):
                return [ps_av.tile([128, 512], F32, tag=f"av{hh}",
                                   name=f"av{hh}_{qb}_{p}")
                        for hh in range(2)]

            def phase_fillers(qb, p):
                f = []
                if qb == 0 and p == 0:
                    def pf(key, fn):
                        prefetch[key] = fn()

                    f.append((1, lambda: (emit_vblock(0, xv0_t),
                                          pf("xk2", lambda: emit_kdma(2)))))
                    f.append((3, lambda: (emit_kt(1, xk1_t),
                                          pf("xv1", lambda: emit_vdma(1)))))
                    f.append((5, lambda: (emit_vblock(1, prefetch["xv1"]),
                                          pf("xk3", lambda: emit_kdma(3)))))
                    f.append((7, lambda: (emit_kt(2, prefetch["xk2"]),
                                          pf("xv2", lambda: emit_vdma(2)))))
                    f.append((9, lambda: (emit_vblock(2, prefetch["xv2"]),
                                          pf("xq1", lambda: emit_qdma(1)))))
                    f.append((10, lambda: pf("xv3", lambda: emit_vdma(3))))
                    f.append((11, lambda: emit_kt(3, prefetch["xk3"])))
                    f.append((13, lambda: emit_vblock(3, prefetch["xv3"])))
                elif qb == 0 and p == 1:
                    f.append((5, lambda: emit_qt(1, prefetch["xq1"])))
                else:
                    prev = qb - 1
                    if p == 0:
                        if qb < N_QB - 1:
                            f.append((8, lambda: prefetch.__setitem__(
                                f"xq{qb + 1}", emit_qdma(qb + 1))))
                        for g in range(4):
                            f.append(((7, 9, 11, 13)[g],
                                      lambda g=g: emit_outproj_chunk(prev, g)))
                    else:
                        if qb < N_QB - 1:
                            f.append((4, lambda: emit_qt(
                                qb + 1, prefetch.get(f"xq{qb + 1}"))))
                        for g in range(4):
                            f.append(((2, 6, 9, 12)[g],
                                      lambda g=g: emit_outproj_chunk(prev, 4 + g)))
                return dict(f)

            emit_kt(0)
            emit_qt(0)

            wv_sb = wpool.tile([128, 8, INNER], BF16)
            nc.sync.dma_start(wv_sb[:], wv.rearrange("(c p) m -> p c m", p=128))
            nc.gpsimd.dma_start(v_sb[:, :, :, DIM_HEAD:DIM_HEAD + 1], vones[:])
            pat_sb = wpool.tile([128, 2, 128], BF16)
            nc.gpsimd.dma_start(pat_sb[:], pat4[:])

            xk1_t = emit_kdma(1)
            xv0_t = emit_vdma(0)

            wo_sb = wpool.tile([128, 2, D_MODEL], BF16)
            nc.sync.dma_start(wo_sb[:], wo.rearrange("(c p) d -> p c d", p=128))

            prefetch = {}

            AV_LAG = 4
            phases = [(qb, p) for qb in range(N_QB) for p in range(2)]
            pending = None      # (qb, p, avs, [(kc, e_t)...])
            pending_fin = None  # (qb, p, avsb)

            for qb, p in phases:
                if p == 0:
                    begin_qb(qb)
                avs = new_avs(qb, p)
                fillers = phase_fillers(qb, p)
                eq = []
                for kc in range(N_KC):
                    e_t, stmm, stmm1 = emit_st(qb, p, kc)
                    eq.append((kc, e_t))
                    if kc == 0 and pending is not None:
                        pq, pp, pavs, peq = pending
                        first_av = None
                        for pkc, pe_t in peq:
                            mms = emit_av(pq, pp, pkc, pavs, pe_t)
                            if first_av is None:
                                first_av = mms[0]
                        add_dep_helper(first_av.ins, stmm1.ins, sync=False,
                                       reason="drain AVs after first ST pair")
                        pending_fin = (pq, pp, evict_pair(pq, pp, pavs))
                        pending = None
                    if kc == 5 and pending_fin is not None:
                        fq, fp, favsb = pending_fin
                        finalize_pair(fq, fp, favsb, order_after=stmm)
                        pending_fin = None
                    if kc >= AV_LAG:
                        pkc, pe_t = eq[kc - AV_LAG]
                        emit_av(qb, p, pkc, avs, pe_t)
                    if kc in fillers:
                        fillers[kc]()
                pending = (qb, p, avs, eq[N_KC - AV_LAG:])

            pq, pp, pavs, peq = pending
            for pkc, pe_t in peq:
                emit_av(pq, pp, pkc, pavs, pe_t)
            avsb_tail = evict_pair(pq, pp, pavs)
            for wi in range(16):
                nc.tensor.matmul(warm_ps[0:64, 0:64], warm_sb[:, 0:64],
                                 warm_sb[:, 0:64], start=True, stop=True)
            finalize_pair(pq, pp, avsb_tail)
            for wi in range(16):
                nc.tensor.matmul(warm_ps[0:64, 0:64], warm_sb[:, 0:64],
                                 warm_sb[:, 0:64], start=True, stop=True)
            for idx in range(8):
                emit_outproj_chunk(N_QB - 1, idx, tail=True)
                if idx in (1, 3, 5):
                    for wi in range(6):
                        nc.tensor.matmul(warm_ps[0:64, 0:64],
                                         warm_sb[:, 0:64],
                                         warm_sb[:, 0:64], start=True,
                                         stop=True)
    nc.compile()
    return nc


_NC_CACHE = None


def _get_nc():
    global _NC_CACHE
    if _NC_CACHE is None:
        _NC_CACHE = build_nc()
    return _NC_CACHE


def _make_pat4():
    pat = np.zeros((128, 2, 128), np.float32)
    for p in range(2):
        for hh in range(2):
            pat[32 * (2 * p + hh), p, hh * 64:(hh + 1) * 64] = 1.0
    return pat.astype(ml_dtypes.bfloat16)


def _bf16(x):
    return np.ascontiguousarray(np.asarray(x, np.float32)).astype(
        ml_dtypes.bfloat16)


def make_in_maps(query, key, value, Wq, Wk, Wv, Wo):
    query = np.asarray(query, np.float32)
    key = np.asarray(key, np.float32)
    value = np.asarray(value, np.float32)
    vones = np.ones((128, N_KC, HEADS_PER_CORE, 1), np.float32).astype(ml_dtypes.bfloat16)
    pat4 = _make_pat4()
    in_maps = []
    for c in range(N_CORES):
        b = c // 4
        hg = c % 4
        cols = slice(hg * INNER, (hg + 1) * INNER)
        in_maps.append({
            "xqt": _bf16(np.asarray(query[b]).T),
            "xkt": _bf16(np.asarray(key[b]).T),
            "xvt": _bf16(np.asarray(value[b]).T),
            "wq": _bf16(np.asarray(Wq[:, cols])),
            "wk": _bf16(np.asarray(Wk[:, cols])),
            "wv": _bf16(np.asarray(Wv[:, cols])),
            "wo": _bf16(np.asarray(Wo[cols, :])),
            "vones": vones,
            "pat4": pat4,
        })
    return in_maps


def kernel(query, key, value, Wq, Wk, Wv, Wo, bo, _trace=False, _trace_cores=None):
    nc = _get_nc()
    in_maps = make_in_maps(query, key, value, Wq, Wk, Wv, Wo)
    res = bass_utils.run_bass_kernel_spmd(
        nc, in_maps, core_ids=list(range(N_CORES)), trace=_trace,
        trace_cores=_trace_cores)
    out = np.zeros((B, N, D_MODEL), np.float32)
    for c in range(N_CORES):
        out[c // 4] += np.asarray(res.results[c]["out"], np.float32)
    out += np.asarray(bo, np.float32)[None, None, :]
    if _trace:
        return out, res
    return out



# revision 23
# speedup vs baseline: 1.4095x; 1.0054x over previous
"""Multi-head attention (B=2, N=2048, d_model=1024, 16 heads x 64) on 8
Trainium2 NeuronCores.

Sharding: batch x head-group. Core c handles batch b = c//4 and heads
4*(c%4) .. 4*(c%4)+3. Projection weights are column-sliced (rows for Wo) so
each core computes q/k/v projections only for its 4 heads, full attention
for those heads, and a partial output projection. The host sums the four
partial outputs per batch (tensor-parallel reduce on to_out) and adds bo.

Device kernel (per core), matmuls in fp32r (rne-11 mantissa):
  qT/kT : projections producing [head-dim, seq] (lhsT = W chunk)
  v     : natural [seq, head-dim] with a ones column folded in (M=65)
  ST    : k^T q per head -> scores^T [keys, queries]; K=64 row-tile PAIRS
          (two heads concurrently on PE tiles T0/T8)
  E     : exp(ST * scale) via ScalarE eviction PSUM->SBUF (the wall:
          ~1 elem/lane/cycle regardless of dtype)
  AV    : [v|ones]^T @ E -> [65, 512]: rows 0-63 = O^T, row 64 = denom
  norm  : one batched reciprocal per query block, denom broadcast via a
          K=4 pattern matmul, DVE multiply into O^T
  out   : O^T-as-lhsT @ Wo slice -> partial [2048, 1024]
"""

import ml_dtypes
import numpy as np

import concourse.mybir as mybir
import concourse.tile as tile
from concourse import bacc
from concourse import bass_utils
from concourse.tile_rust import add_dep_helper

F32 = mybir.dt.float32
BF16 = mybir.dt.bfloat16
F32R = mybir.dt.float32r
E_DT = mybir.dt.bfloat16
EXP = mybir.ActivationFunctionType.Exp

B = 2
N = 2048
D_MODEL = 1024
NHEAD = 16
DIM_HEAD = 64
SCALE = DIM_HEAD ** (-0.5)
N_CORES = 8
HEADS_PER_CORE = 4          # 2 pairs
INNER = HEADS_PER_CORE * DIM_HEAD  # 256

QB = 512                    # query block
N_QB = N // QB              # 4
N_KC = N // 128             # 16 key chunks


def build_nc():
    nc = bacc.Bacc("TRN2", target_bir_lowering=False, debug=False,
                   num_devices=N_CORES)
    xqt = nc.dram_tensor("xqt", [D_MODEL, N], BF16, kind="ExternalInput").ap()
    xkt = nc.dram_tensor("xkt", [D_MODEL, N], BF16, kind="ExternalInput").ap()
    xvt = nc.dram_tensor("xvt", [D_MODEL, N], BF16, kind="ExternalInput").ap()
    wq = nc.dram_tensor("wq", [D_MODEL, INNER], BF16, kind="ExternalInput").ap()
    wk = nc.dram_tensor("wk", [D_MODEL, INNER], BF16, kind="ExternalInput").ap()
    wv = nc.dram_tensor("wv", [D_MODEL, INNER], BF16, kind="ExternalInput").ap()
    wo = nc.dram_tensor("wo", [INNER, D_MODEL], BF16, kind="ExternalInput").ap()
    vones = nc.dram_tensor("vones", [128, N_KC, HEADS_PER_CORE, 1], BF16,
                           kind="ExternalInput").ap()
    # bc pattern: pat4[k, p, m] = 1 where head k owns output rows m in pair p
    pat4 = nc.dram_tensor("pat4", [128, 2, 128], BF16, kind="ExternalInput").ap()
    out = nc.dram_tensor("out", [N, D_MODEL], BF16, kind="ExternalOutput").ap()

    with tile.TileContext(nc) as tc:
        with (
            tc.tile_pool(name="wpool", bufs=1) as wpool,
            tc.tile_pool(name="persist", bufs=1) as persist,
            tc.tile_pool(name="xin", bufs=6) as xin,
            tc.tile_pool(name="ering", bufs=9) as ering,
            tc.tile_pool(name="stage", bufs=3) as stage,
            tc.tile_pool(name="ps_st", bufs=2, space="PSUM") as ps_st,
            tc.tile_pool(name="ps_av", bufs=1, space="PSUM") as ps_av,
            tc.tile_pool(name="ps_misc", bufs=2, space="PSUM") as ps_misc,
        ):
            # ---- PE pre-warm: keep HAM busy so real matmuls start at
            # full clock. Dummy matmuls on a tiny scratch tile. ----
            warm_sb = wpool.tile([128, 64], BF16)
            nc.vector.memset(warm_sb[:], 0.0)
            warm_ps = ps_misc.tile([128, 64], F32, tag="mp", name="warmps")
            for wi in range(32):
                nc.tensor.matmul(warm_ps[0:64, 0:64], warm_sb[:, 0:64],
                                 warm_sb[:, 0:64], start=True, stop=True)

            # ---- weights on sync queue, ordered by first use ----
            wk_sb = wpool.tile([128, 8, INNER], BF16)
            wk_r = wk.rearrange("(c p) m -> p c m", p=128)
            nc.sync.dma_start(wk_sb[:, 0:4, :], wk_r[:, 0:4, :])
            nc.gpsimd.dma_start(wk_sb[:, 4:8, :], wk_r[:, 4:8, :])
            wq_sb = wpool.tile([128, 8, INNER], BF16)
            wq_r = wq.rearrange("(c p) m -> p c m", p=128)
            nc.sync.dma_start(wq_sb[:, 0:4, :], wq_r[:, 0:4, :])
            nc.gpsimd.dma_start(wq_sb[:, 4:8, :], wq_r[:, 4:8, :])

            qt_sb = persist.tile([128, 2, N], BF16)
            kt_sb = persist.tile([128, 2, N], BF16)
            v_sb = persist.tile([128, N_KC, HEADS_PER_CORE, DIM_HEAD + 1], BF16)
            ot_sb = persist.tile([128, 2, N], BF16)

            xqt_r = xqt.rearrange("(c p) n -> p c n", p=128)
            xkt_r = xkt.rearrange("(c p) n -> p c n", p=128)
            xvt_r = xvt.rearrange("(c p) n -> p c n", p=128)

            def emit_kdma(n):
                ns = slice(n * QB, (n + 1) * QB)
                xk_t = xin.tile([128, 8, QB], BF16, tag="xin", name=f"xk_{n}")
                for cc in range(4):
                    eng = (nc.sync, nc.gpsimd)[cc % 2]
                    eng.dma_start(xk_t[:, 2 * cc:2 * cc + 2, :],
                                  xkt_r[:, 2 * cc:2 * cc + 2, ns])
                return xk_t

            def emit_kt(n, xk_t=None):
                ns = slice(n * QB, (n + 1) * QB)
                if xk_t is None:
                    xk_t = emit_kdma(n)
                for m in range(2):
                    pk = ps_misc.tile([128, QB], F32, tag="mp", name=f"pk{n}{m}")
                    for c in range(8):
                        nc.tensor.matmul(
                            pk[:], wk_sb[:, c, m * 128:(m + 1) * 128],
                            xk_t[:, c, :], start=(c == 0), stop=(c == 7))
                    nc.vector.tensor_copy(kt_sb[:, m, ns], pk[:])

            def emit_qdma(n):
                ns = slice(n * QB, (n + 1) * QB)
                xq_t = xin.tile([128, 8, QB], BF16, tag="xin", name=f"xq_{n}")
                for cc in range(4):
                    eng = (nc.sync, nc.gpsimd)[cc % 2]
                    eng.dma_start(xq_t[:, 2 * cc:2 * cc + 2, :],
                                  xqt_r[:, 2 * cc:2 * cc + 2, ns])
                return xq_t

            def emit_qt(n, xq_t=None):
                ns = slice(n * QB, (n + 1) * QB)
                if xq_t is None:
                    xq_t = emit_qdma(n)
                for m in range(2):
                    pq = ps_misc.tile([128, QB], F32, tag="mp", name=f"pq{n}{m}")
                    for c in range(8):
                        nc.tensor.matmul(
                            pq[:], wq_sb[:, c, m * 128:(m + 1) * 128],
                            xq_t[:, c, :], start=(c == 0), stop=(c == 7))
                    nc.vector.tensor_copy(qt_sb[:, m, ns], pq[:])

            def emit_vdma(n):
                ns = slice(n * QB, (n + 1) * QB)
                xv_t = xin.tile([128, 8, QB], BF16, tag="xin", name=f"xv_{n}")
                for cc in range(2):
                    eng = (nc.gpsimd, nc.sync)[cc % 2]
                    eng.dma_start(xv_t[:, 4 * cc:4 * cc + 4, :],
                                  xvt_r[:, 4 * cc:4 * cc + 4, ns])
                return xv_t

            def emit_vblock(n, xv_t=None):
                ns = slice(n * QB, (n + 1) * QB)
                if xv_t is None:
                    xv_t = emit_vdma(n)
                for kci in range(4):
                    kc = n * 4 + kci
                    kcs = slice(kci * 128, (kci + 1) * 128)
                    pva = ps_misc.tile([128, INNER], F32, tag="mp",
                                       padded_shape=[128, 512], name=f"pva{kc}")
                    pvb = ps_misc.tile([128, INNER], F32, tag="mp",
                                       padded_shape=[128, 512], name=f"pvb{kc}")
                    for c in range(8):
                        nc.tensor.matmul(
                            pva[:], xv_t[0:64, c, kcs],
                            wv_sb[0:64, c, :], start=(c == 0), stop=(c == 7))
                        nc.tensor.matmul(
                            pvb[:], xv_t[64:128, c, kcs],
                            wv_sb[64:128, c, :], start=(c == 0), stop=(c == 7))
                    va_sb = stage.tile([128, INNER], F32, tag="vasb",
                                       name=f"va{kc}", bufs=2)
                    nc.vector.tensor_copy(va_sb[:], pva[:])
                    nc.vector.scalar_tensor_tensor(
                        v_sb[:, kc, :, 0:DIM_HEAD],
                        pvb[:].rearrange("p (h d) -> p h d", h=HEADS_PER_CORE),
                        1.0,
                        va_sb[:].rearrange("p (h d) -> p h d", h=HEADS_PER_CORE),
                        mybir.AluOpType.mult, mybir.AluOpType.add)

            def emit_outproj_chunk(qb, idx, tail=False):
                qc = qb * 4 + idx // 2
                dc = idx % 2
                cs = slice(qc * 128, (qc + 1) * 128)
                op = ps_misc.tile([128, 512], F32, tag="mp", name=f"op{qc}{dc}")
                for ic in range(2):
                    nc.tensor.matmul(
                        op[:], ot_sb[:, ic, cs],
                        wo_sb[:, ic, dc * 512:(dc + 1) * 512],
                        start=(ic == 0), stop=(ic == 1))
                o_stage = stage.tile([128, 512], BF16, tag="ostage",
                                     name=f"ost{qc}{dc}", bufs=2)
                if tail and (idx % 2 == 0):
                    nc.scalar.copy(o_stage[:], op[:])
                else:
                    nc.vector.tensor_copy(o_stage[:], op[:])
                deng = (nc.sync, nc.gpsimd)[dc]
                deng.dma_start(out[cs, dc * 512:(dc + 1) * 512], o_stage[:])

            qb_state = {}

            def emit_st(qb, p, kc):
                qs = slice(qb * QB, (qb + 1) * QB)
                ks = slice(kc * 128, (kc + 1) * 128)
                st = ps_st.tile([128, 1024], F32, tag="st", name=f"st{qb}{p}{kc}")
                mm0 = nc.tensor.matmul(st[:, 0:512], kt_sb[0:64, p, ks],
                                       qt_sb[0:64, p, qs], start=True, stop=True)
                mm1 = nc.tensor.matmul(st[:, 512:1024], kt_sb[64:128, p, ks],
                                 qt_sb[64:128, p, qs], start=True, stop=True)
                e_t = ering.tile([128, 2, 512], E_DT, tag="e",
                                 name=f"e{qb}{p}{kc}")
                nc.scalar.activation(
                    e_t[:], st[:].rearrange("p (h n) -> p h n", h=2),
                    EXP, scale=float(SCALE))
                return e_t, mm0, mm1

            def emit_av(qb, p, kc, avs, e_t):
                mms = []
                for hh in range(2):
                    mms.append(nc.tensor.matmul(
                        avs[hh][0:DIM_HEAD + 1, :],
                        v_sb[:, kc, 2 * p + hh, :], e_t[:, hh, :],
                        start=(kc == 0), stop=(kc == N_KC - 1)))
                return mms

            def evict_pair(qb, p, avs):
                den4 = qb_state[qb]["den4"]
                avsb = []
                for hh in range(2):
                    a_sb = stage.tile([DIM_HEAD + 1, 512], F32, tag="avsb",
                                      name=f"avsb{qb}_{p}_{hh}", bufs=4)
                    nc.vector.tensor_copy(a_sb[:], avs[hh][0:DIM_HEAD + 1, :])
                    k32 = 32 * (2 * p + hh)
                    nc.vector.tensor_copy(den4[k32:k32 + 1, :],
                                          a_sb[DIM_HEAD:DIM_HEAD + 1, :])
                    avsb.append(a_sb)
                return avsb

            def finalize_pair(qb, p, avsb, order_after=None):
                den4 = qb_state[qb]["den4"]
                qs = slice(qb * QB, (qb + 1) * QB)
                rec = stage.tile([128, 512], F32, tag="rec",
                                 name=f"rec{qb}{p}", bufs=2)
                nc.vector.reciprocal_approx_fast(rec[:], den4[:])
                recr = stage.tile([128, 512], BF16, tag="recr",
                                  name=f"recr{qb}{p}", bufs=2)
                nc.vector.tensor_copy(recr[:], rec[:])
                bc = ps_misc.tile([128, 512], F32, tag="mp", name=f"bc{qb}{p}")
                bcmm = nc.tensor.matmul(bc[:], pat_sb[:, p, :], recr[:],
                                        start=True, stop=True)
                if order_after is not None:
                    add_dep_helper(order_after.ins, bcmm.ins, sync=False,
                                   reason="hold bc behind ST stream")
                for hh in range(2):
                    nc.vector.tensor_mul(
                        ot_sb[hh * 64:(hh + 1) * 64, p, qs],
                        avsb[hh][0:DIM_HEAD, :],
                        bc[hh * 64:(hh + 1) * 64, :])


            def begin_qb(qb):
                den4 = stage.tile([128, 512], F32, tag="den4", name=f"den{qb}",
                                  bufs=1)
                nc.gpsimd.memset(den4[:], 1.0)
                qb_state[qb] = dict(den4=den4)

            def new_avs(qb, p):
                return [ps_av.tile([128, 512], F32, tag=f"av{hh}",
                                   name=f"av{hh}_{qb}_{p}")
                        for hh in range(2)]

            def phase_fillers(qb, p):
                f = []
                if qb == 0 and p == 0:
                    def pf(key, fn):
                        prefetch[key] = fn()

                    f.append((1, lambda: (emit_vblock(0, xv0_t),
                                          pf("xv1", lambda: emit_vdma(1)))))
                    f.append((3, lambda: (emit_kt(1, xk1_t),
                                          pf("xk2", lambda: emit_kdma(2)))))
                    f.append((5, lambda: (emit_vblock(1, prefetch["xv1"]),
                                          pf("xv2", lambda: emit_vdma(2)))))
                    f.append((7, lambda: (emit_kt(2, prefetch["xk2"]),
                                          pf("xk3", lambda: emit_kdma(3)))))
                    f.append((9, lambda: (emit_vblock(2, prefetch["xv2"]),
                                          pf("xv3", lambda: emit_vdma(3)),
                                          pf("xq1", lambda: emit_qdma(1)))))
                    f.append((11, lambda: emit_kt(3, prefetch["xk3"])))
                    f.append((13, lambda: emit_vblock(3, prefetch["xv3"])))
                elif qb == 0 and p == 1:
                    f.append((5, lambda: emit_qt(1, prefetch["xq1"])))
                else:
                    prev = qb - 1
                    if p == 0:
                        if qb < N_QB - 1:
                            f.append((8, lambda: prefetch.__setitem__(
                                f"xq{qb + 1}", emit_qdma(qb + 1))))
                        for g in range(4):
                            f.append(((7, 9, 11, 13)[g],
                                      lambda g=g: emit_outproj_chunk(prev, g)))
                    else:
                        if qb < N_QB - 1:
                            f.append((4, lambda: emit_qt(
                                qb + 1, prefetch.get(f"xq{qb + 1}"))))
                        for g in range(4):
                            f.append(((2, 6, 9, 12)[g],
                                      lambda g=g: emit_outproj_chunk(prev, 4 + g)))
                return dict(f)

            emit_kt(0)
            emit_qt(0)

            wv_sb = wpool.tile([128, 8, INNER], BF16)
            nc.sync.dma_start(wv_sb[:], wv.rearrange("(c p) m -> p c m", p=128))
            nc.gpsimd.dma_start(v_sb[:, :, :, DIM_HEAD:DIM_HEAD + 1], vones[:])
            pat_sb = wpool.tile([128, 2, 128], BF16)
            nc.gpsimd.dma_start(pat_sb[:], pat4[:])

            xk1_t = emit_kdma(1)
            xv0_t = emit_vdma(0)

            wo_sb = wpool.tile([128, 2, D_MODEL], BF16)
            nc.sync.dma_start(wo_sb[:], wo.rearrange("(c p) d -> p c d", p=128))

            prefetch = {}

            AV_LAG = 4
            phases = [(qb, p) for qb in range(N_QB) for p in range(2)]
            pending = None      # (qb, p, avs, [(kc, e_t)...])
            pending_fin = None  # (qb, p, avsb)

            for qb, p in phases:
                if p == 0:
                    begin_qb(qb)
                avs = new_avs(qb, p)
                fillers = phase_fillers(qb, p)
                eq = []
                for kc in range(N_KC):
                    e_t, stmm, stmm1 = emit_st(qb, p, kc)
                    eq.append((kc, e_t))
                    if kc == 0 and pending is not None:
                        pq, pp, pavs, peq = pending
                        first_av = None
                        for pkc, pe_t in peq:
                            mms = emit_av(pq, pp, pkc, pavs, pe_t)
                            if first_av is None:
                                first_av = mms[0]
                        add_dep_helper(first_av.ins, stmm1.ins, sync=False,
                                       reason="drain AVs after first ST pair")
                        pending_fin = (pq, pp, evict_pair(pq, pp, pavs))
                        pending = None
                    if kc == 5 and pending_fin is not None:
                        fq, fp, favsb = pending_fin
                        finalize_pair(fq, fp, favsb, order_after=stmm)
                        pending_fin = None
                    if kc >= AV_LAG:
                        pkc, pe_t = eq[kc - AV_LAG]
                        emit_av(qb, p, pkc, avs, pe_t)
                    if kc in fillers:
                        fillers[kc]()
                pending = (qb, p, avs, eq[N_KC - AV_LAG:])

            pq, pp, pavs, peq = pending
            for pkc, pe_t in peq:
                emit_av(pq, pp, pkc, pavs, pe_t)
            avsb_tail = evict_pair(pq, pp, pavs)
            for wi in range(20):
                nc.tensor.matmul(warm_ps[0:64, 0:64], warm_sb[:, 0:64],
                                 warm_sb[:, 0:64], start=True, stop=True)
            finalize_pair(pq, pp, avsb_tail)
            for idx in range(8):
                emit_outproj_chunk(N_QB - 1, idx, tail=True)
    nc.compile()
    return nc


_NC_CACHE = None


def _get_nc():
    global _NC_CACHE
    if _NC_CACHE is None:
        _NC_CACHE = build_nc()
    return _NC_CACHE


def _make_pat4():
    pat = np.zeros((128, 2, 128), np.float32)
    for p in range(2):
        for hh in range(2):
            pat[32 * (2 * p + hh), p, hh * 64:(hh + 1) * 64] = 1.0
    return pat.astype(ml_dtypes.bfloat16)


def _bf16(x):
    return np.ascontiguousarray(np.asarray(x, np.float32)).astype(
        ml_dtypes.bfloat16)


def make_in_maps(query, key, value, Wq, Wk, Wv, Wo):
    query = np.asarray(query, np.float32)
    key = np.asarray(key, np.float32)
    value = np.asarray(value, np.float32)
    vones = np.ones((128, N_KC, HEADS_PER_CORE, 1), np.float32).astype(ml_dtypes.bfloat16)
    pat4 = _make_pat4()
    in_maps = []
    for c in range(N_CORES):
        b = c // 4
        hg = c % 4
        cols = slice(hg * INNER, (hg + 1) * INNER)
        in_maps.append({
            "xqt": _bf16(np.asarray(query[b]).T),
            "xkt": _bf16(np.asarray(key[b]).T),
            "xvt": _bf16(np.asarray(value[b]).T),
            "wq": _bf16(np.asarray(Wq[:, cols])),
            "wk": _bf16(np.asarray(Wk[:, cols])),
            "wv": _bf16(np.asarray(Wv[:, cols])),
            "wo": _bf16(np.asarray(Wo[cols, :])),
            "vones": vones,
            "pat4": pat4,
        })
    return in_maps


def kernel(query, key, value, Wq, Wk, Wv, Wo, bo, _trace=False, _trace_cores=None):
    nc = _get_nc()
    in_maps = make_in_maps(query, key, value, Wq, Wk, Wv, Wo)
    res = bass_utils.run_bass_kernel_spmd(
        nc, in_maps, core_ids=list(range(N_CORES)), trace=_trace,
        trace_cores=_trace_cores)
    out = np.zeros((B, N, D_MODEL), np.float32)
    for c in range(N_CORES):
        out[c // 4] += np.asarray(res.results[c]["out"], np.float32)
    out += np.asarray(bo, np.float32)[None, None, :]
    if _trace:
        return out, res
    return out

